# revision 1
# baseline (speedup 1.0000x reference)
"""Bidirectional cross-attention Trainium2 kernel.

Sharding: (batch, head) units. B=2, H=12 -> 24 units over 8 cores:
core c handles batch b = c // 4 and heads 3*(c%4) .. 3*(c%4)+2.
Each core computes the full attention for its 3 heads plus the partial
output projections; the host sums the per-core partial projections
(the "all-reduce after the output projections"), transposes back, adds
biases and concatenates the two branches.

All host-side work is pure data marshalling: transpose/slice/scale-fold
on inputs, summation/transpose/bias on outputs. All matmuls, softmaxes
and projections run on-device.
"""

import os
import sys
from contextlib import ExitStack

import numpy as np

sys.path.insert(0, "/opt/trn_rl_repo")

import ml_dtypes  # noqa: E402

import concourse.bass as bass  # noqa: E402
import concourse.tile as tile  # noqa: E402
from concourse import bacc, mybir  # noqa: E402
from concourse import bass_utils  # noqa: E402

# ---------------------------------------------------------------- constants
P = 128          # partitions
C = 768          # channels
CB = C // P      # 6 channel blocks
NH = 3           # heads per core
D2 = 128         # qk dims per head (2*HEAD_DIM)
DH = 64          # v dims per head
QW = NH * D2     # 384
VW = NH * DH     # 192
H = 12
B = 2
N_CORES = 8
SCALE = DH ** -0.5

BF = mybir.dt.bfloat16
F32 = mybir.dt.float32

_PROG_CACHE: dict[int, "bacc.Bacc"] = {}


def _build_program(NT: int) -> "bacc.Bacc":
    """Build+schedule+compile the per-core Bass program (SPMD: same program
    on all 8 cores, per-core data differs)."""
    NCH = NT // P      # 128-row chunks
    N5 = NT // 512     # 512-col chunks
    N10 = NT // 1024   # 1024-col chunks

    nc = bacc.Bacc(
        "TRN2",
        target_bir_lowering=False,
        debug=False,
        num_devices=N_CORES,
    )

    xT_d = nc.dram_tensor("xT", [C, NT], BF, kind="ExternalInput").ap()
    srcT_d = nc.dram_tensor("srcT", [C, NT], BF, kind="ExternalInput").ap()
    qkw_d = nc.dram_tensor("qk_wT", [C, QW], BF, kind="ExternalInput").ap()
    qksw_d = nc.dram_tensor("qks_wT", [C, QW], BF, kind="ExternalInput").ap()
    vw_d = nc.dram_tensor("v_wT", [C, VW], BF, kind="ExternalInput").ap()
    vsw_d = nc.dram_tensor("vs_wT", [C, VW], BF, kind="ExternalInput").ap()
    pjw_d = nc.dram_tensor("projT", [VW, C], BF, kind="ExternalInput").ap()
    pjsw_d = nc.dram_tensor("projsT", [VW, C], BF, kind="ExternalInput").ap()
    ident_d = nc.dram_tensor("ident", [P, P], BF, kind="ExternalInput").ap()
    oy_d = nc.dram_tensor("out_y", [C, NT], BF, kind="ExternalOutput").ap()
    oys_d = nc.dram_tensor("out_ys", [C, NT], BF, kind="ExternalOutput").ap()

    with tile.TileContext(nc) as tc, ExitStack() as ctx:
        sb = ctx.enter_context(tc.tile_pool(name="sb", bufs=1, space="SBUF"))
        ps = ctx.enter_context(tc.tile_pool(name="ps", bufs=2, space="PSUM"))

        # ---------------- input loads
        def load_rows(dram, rows, cols, tag):
            tiles = []
            for i in range(rows // P):
                t = sb.tile([P, cols], BF, tag=tag, bufs=rows // P,
                            name=f"{tag}{i}")
                nc.sync.dma_start(t[:], dram[P * i:P * (i + 1), :])
                tiles.append(t)
            return tiles

        # x-side tensors first so phase A starts as early as possible
        xt = load_rows(xT_d, C, NT, "xt")
        qkw = load_rows(qkw_d, C, QW, "qkw")

        ones64 = sb.tile([P, DH], BF, tag="ones64")
        nc.gpsimd.memset(ones64[:], 1.0)
        zb = sb.tile([P, 1], F32, tag="zb")
        nc.gpsimd.memset(zb[:], 0.0)
        ident = sb.tile([P, P], BF, tag="ident")
        nc.sync.dma_start(ident[:], ident_d[:])

        # ---------------- phase A: per-head transposed QK projections
        # qkt[u][d2, n] = sum_c qk_wT[c, 128u+d2] * xT[c, n]
        def gen_qkt_head(act_tiles, w_tiles, tag, u, copy_eng):
            t = sb.tile([P, NT], BF, tag=tag, bufs=NH, name=f"{tag}{u}")
            for jj in range(N10):
                pst = ps.tile([P, 1024], F32, tag="ps_sim",
                              name=f"ps_{tag}{u}_{jj}")
                for h2 in range(2):
                    lo = 1024 * jj + 512 * h2
                    for cb in range(CB):
                        nc.tensor.matmul(
                            pst[:, 512 * h2:512 * (h2 + 1)],
                            lhsT=w_tiles[cb][:, D2 * u:D2 * (u + 1)],
                            rhs=act_tiles[cb][:, lo:lo + 512],
                            start=(cb == 0), stop=(cb == CB - 1),
                        )
                nc.vector.tensor_copy(
                    t[:, 1024 * jj:1024 * (jj + 1)], pst[:])
            return t

        # filler variant: the head projection split into N5 closures of one
        # 512-column psum group each, to be woven between a pass's row-blocks
        # (engine queues are strict FIFO — work must be EMITTED interleaved
        # to fill PE idle slots during ACT-bound passes).
        def qkt_fillers(act_tiles, w_tiles, tag, u):
            t = sb.tile([P, NT], BF, tag=tag, bufs=NH, name=f"{tag}{u}")

            def mk(q):
                def f():
                    pst = ps.tile([P, 512], F32, tag="ps_small",
                                  name=f"psq_{tag}{u}_{q}")
                    for cb in range(CB):
                        nc.tensor.matmul(
                            pst[:],
                            lhsT=w_tiles[cb][:, D2 * u:D2 * (u + 1)],
                            rhs=act_tiles[cb][:, 512 * q:512 * (q + 1)],
                            start=(cb == 0), stop=(cb == CB - 1),
                        )
                    nc.vector.tensor_copy(t[:, 512 * q:512 * (q + 1)], pst[:])
                return f
            return t, [mk(q) for q in range(N5)]

        # ---------------- phase A: v in natural layout, ones-augmented.
        # vax[:, u*NCH*65 + 65*i : +64] = v[128i:128(i+1), 64u:64u+64]
        # column 64 of each 65-block is 1.0 (gives softmax denominator row).
        def gen_vaug(act_tiles, w_tiles, tag, chunks=None, tile_=None):
            if chunks is None:
                chunks = range(NCH)
            vax = tile_ if tile_ is not None else sb.tile(
                [P, NH * NCH * 65], BF, tag=tag, name=tag)
            vr = vax.rearrange("p (u i e) -> p u i e", u=NH, e=65)
            for i in chunks:
                psv = ps.tile([P, VW], F32, tag="ps_small",
                              name=f"psv_{tag}{i}")
                for cb in range(CB):
                    nc.tensor.matmul(
                        psv[:],
                        lhsT=act_tiles[cb][:, P * i:P * (i + 1)],
                        rhs=w_tiles[cb][:],
                        start=(cb == 0), stop=(cb == CB - 1),
                    )
                nc.vector.tensor_copy(
                    vr[:, :, i, 0:DH],
                    psv.rearrange("p (u e) -> p u e", e=DH),
                )
                nc.gpsimd.memset(vr[:, :, i, DH:65], 1.0)
            return vax

        # src-side loads emitted now so DMA streams them right after x-side
        st = load_rows(srcT_d, C, NT, "st")
        qksw = load_rows(qksw_d, C, QW, "qksw")
        vw = load_rows(vw_d, C, VW, "vw")
        vsw = load_rows(vsw_d, C, VW, "vsw")

        # head 0 of both sides first: the first attention pass starts as
        # early as possible. Only the first few vax chunks are emitted up
        # front; the rest ride inside pass 1 (positions keep each chunk's
        # producer ahead of its consumer in the PE FIFO).
        qkt = [None] * NH
        qkst = [None] * NH
        qkt[0] = gen_qkt_head(xt, qkw, "qkt", 0, "vector")
        vax = sb.tile([P, NH * NCH * 65], BF, tag="vax", name="vax")
        gen_vaug(xt, vw, "vax", tile_=vax)
        qkst[0] = gen_qkt_head(st, qksw, "qkst", 0, "vector")

        pj1 = sb.tile([P, C], BF, tag="pj1")
        nc.sync.dma_start(pj1[:], pjw_d[0:P, :])
        pj2 = sb.tile([DH, C], BF, tag="pj2")
        nc.sync.dma_start(pj2[:], pjw_d[P:VW, :])
        pjs1 = sb.tile([P, C], BF, tag="pjs1")
        nc.sync.dma_start(pjs1[:], pjsw_d[0:P, :])
        pjs2 = sb.tile([DH, C], BF, tag="pjs2")
        nc.sync.dma_start(pjs2[:], pjsw_d[P:VW, :])

        # ---------------- head stacks for the output projections
        Y01 = sb.tile([P, NT], BF, tag="Y01")
        Y2 = sb.tile([DH, NT], BF, tag="Y2")
        YS01 = sb.tile([P, NT], BF, tag="YS01")
        YS2 = sb.tile([DH, NT], BF, tag="YS2")

        def stack_slice(s01, s2, u):
            if u == 0:
                return s01[0:DH, :]
            if u == 1:
                return s01[DH:P, :]
            return s2[0:DH, :]

        # ---------------- one attention pass (one softmax direction)
        # qa/qb: [d2, token] transposed qk tensors; for row-chunk i of
        # qa-tokens: psum = qa_chunk^T @ qb  (sim block), ES = exp(psum),
        # acc[0:64] += va_head^T @ ES  (raw weighted values),
        # acc[64]   += ones^T @ ES     (softmax denominator).
        # Then dst = acc[0:64] * (1/acc[64]) broadcast.
        # emits the matmul/exp body of one pass; returns a closure that emits
        # the normalization. The caller emits that closure AFTER the next
        # pass's matmuls: engine queues are strict FIFO, so norm instructions
        # (whose head waits on the slow reciprocal) must sit behind the next
        # pass's matmuls in the PE queue, not in front of them.
        def attention_pass(u, qa, qb, va, dst, pname, fillers=(),
                           positions=None, tail=False):
            fillers = list(fillers)
            n_iters = N10 * NCH
            if positions is None:
                step = max(2, n_iters // (len(fillers) + 1)) if fillers else 0
                positions = [step * (j + 1) for j in range(len(fillers))]
            positions = list(positions)
            it = 0
            accS = sb.tile([65, NT], BF, tag="accS", bufs=2,
                           name=f"accS_{pname}")
            for half in range(N10):
                acc = ps.tile([65, 1024], F32, tag="ps_acc", bufs=1,
                              name=f"acc_{pname}_{half}")
                for i in range(NCH):
                    est = sb.tile([P, 1024], BF, tag="es", bufs=3,
                                  name=f"es_{pname}_{half}_{i}")
                    pst = ps.tile([P, 1024], F32, tag="ps_sim",
                                  name=f"ps_{pname}_{half}_{i}")
                    for h2 in range(2):
                        lo = 1024 * half + 512 * h2
                        nc.tensor.matmul(
                            pst[:, 512 * h2:512 * (h2 + 1)],
                            lhsT=qa[:, P * i:P * (i + 1)],
                            rhs=qb[:, lo:lo + 512],
                            start=True, stop=True,
                        )
                    nc.scalar.activation(
                        est[:], pst[:],
                        mybir.ActivationFunctionType.Exp, bias=zb[:],
                    )
                    vsl = va[:, u * NCH * 65 + 65 * i:
                             u * NCH * 65 + 65 * (i + 1)]
                    for k in range(2):
                        nc.tensor.matmul(
                            acc[:, 512 * k:512 * (k + 1)],
                            lhsT=vsl,
                            rhs=est[:, 512 * k:512 * (k + 1)],
                            start=(i == 0), stop=(i == NCH - 1),
                        )
                    it += 1
                    while fillers and positions and it >= positions[0]:
                        positions.pop(0)
                        fillers.pop(0)()
                # copy to SBUF promptly: releases the PSUM accumulator;
                # normalization runs off the critical path. On the final
                # pass the last copy goes to ACT (idle once exps finish).
                if tail and half == N10 - 1:
                    nc.scalar.copy(
                        accS[:, 1024 * half:1024 * (half + 1)], acc[:])
                else:
                    nc.vector.tensor_copy(
                        accS[:, 1024 * half:1024 * (half + 1)], acc[:])
            for f in fillers:   # flush any leftovers
                f()

            def norm():
                # dst = accS[0:64] / accS[64].
                # A [1, NT] reciprocal runs on one DVE lane (~13us) — instead
                # spread the row into psum columns with K=1 matmuls, take the
                # reciprocal on [128, NCH] (all lanes, ~0.3us), and broadcast
                # back with step-0-weight matmuls against the identity.
                psc = ps.tile([P, NCH], F32, tag="ps_small",
                              name=f"psc_{pname}")
                for j in range(NCH):
                    nc.tensor.matmul(
                        psc[:, j:j + 1],
                        lhsT=accS[64:65, P * j:P * (j + 1)],
                        rhs=ones64[64:65, 0:1],
                        start=True, stop=True,
                    )
                rcpF = sb.tile([P, NCH], F32, tag="rcpF", bufs=2,
                               name=f"rcpF_{pname}")
                nc.vector.reciprocal(rcpF[:], psc[:])
                rcpT = sb.tile([P, NCH], BF, tag="rcpT", bufs=2,
                               name=f"rcpT_{pname}")
                nc.vector.tensor_copy(rcpT[:], rcpF[:])
                bb = sb.tile([DH, NT], BF, tag="bb", bufs=2,
                             name=f"bb_{pname}")
                for k in range(N5):
                    pso = ps.tile([DH, 512], F32, tag="ps_small",
                                  name=f"psbb_{pname}_{k}")
                    for jj in range(4):
                        j = 4 * k + jj
                        col = rcpT[:, j:j + 1]
                        lhsT_b = bass.AP(col.tensor, col.offset,
                                         [col.ap[0], [0, DH]])
                        nc.tensor.matmul(
                            pso[:, P * jj:P * (jj + 1)], lhsT=lhsT_b,
                            rhs=ident[:], start=True, stop=True,
                        )
                    nc.vector.tensor_copy(bb[:, 512 * k:512 * (k + 1)],
                                          pso[:])
                if tail:
                    for hh in range(N10):
                        sl = slice(1024 * hh, 1024 * (hh + 1))
                        nc.vector.tensor_mul(dst[:, sl], accS[0:DH, sl],
                                             bb[:, sl])
                else:
                    nc.vector.tensor_mul(dst, accS[0:DH, :], bb[:])
            return norm

        # ---------------- partial output projection (3 heads stacked, K=192)
        def oproj(S1, S2, W1, W2, out_d, pname, copy_eng="vector",
                  ccs=None):
            for cc in (range(CB) if ccs is None else ccs):
                for k in range(N5):
                    pso = ps.tile([P, 512], F32, tag="ps_small",
                                  name=f"pso_{pname}_{cc}_{k}")
                    nc.tensor.matmul(
                        pso[:], lhsT=W1[:, P * cc:P * (cc + 1)],
                        rhs=S1[:, 512 * k:512 * (k + 1)],
                        start=True, stop=False,
                    )
                    nc.tensor.matmul(
                        pso[:], lhsT=W2[:, P * cc:P * (cc + 1)],
                        rhs=S2[:, 512 * k:512 * (k + 1)],
                        start=False, stop=True,
                    )
                    stg = sb.tile([P, 512], BF, tag="stg", bufs=4,
                                  name=f"stg_{pname}_{cc}_{k}")
                    if copy_eng == "scalar" or (
                            copy_eng == "both" and (cc + k) % 2 == 0):
                        nc.scalar.copy(stg[:], pso[:])
                    else:
                        nc.vector.tensor_copy(stg[:], pso[:])
                    nc.sync.dma_start(
                        out_d[P * cc:P * (cc + 1), 512 * k:512 * (k + 1)],
                        stg[:],
                    )

        # ---------------- main loop over this core's 3 heads.
        # Each pass's normalization is emitted after the NEXT pass's matmuls
        # (see attention_pass). vas is emitted after u0-pass1 so its PE work
        # fills the queue while ACT runs u0-pass1's exps.
        norms = {}
        # later heads' projections, vas generation and the src-branch output
        # projection all ride INSIDE pass windows as fillers.
        qkt[1], fq1 = qkt_fillers(xt, qkw, "qkt", 1)
        qkst[1], fq2 = qkt_fillers(st, qksw, "qkst", 1)
        qkt[2], fq3 = qkt_fillers(xt, qkw, "qkt", 2)
        qkst[2], fq4 = qkt_fillers(st, qksw, "qkst", 2)
        vas = sb.tile([P, NH * NCH * 65], BF, tag="vas", name="vas")
        fvas = [
            (lambda c0: lambda: gen_vaug(st, vsw, "vas",
                                         chunks=range(c0, c0 + 4),
                                         tile_=vas))(c0)
            for c0 in range(0, NCH, 4)
        ]
        fosrc = [
            (lambda cc: lambda: oproj(YS01, YS2, pjs1, pjs2, oys_d,
                                      f"osrc{cc}", copy_eng="vector",
                                      ccs=[cc]))(cc)
            for cc in range(CB)
        ]

        # ---- pass-1 block (all heads; uses vax only)
        fill_p1 = {0: fq1 + fq2, 1: fq3 + fq4, 2: fvas}
        for u in range(NH):
            norms[("p1", u)] = attention_pass(
                u, qkt[u], qkst[u], vax, stack_slice(YS01, YS2, u),
                f"p1u{u}", fillers=fill_p1[u])
            if u >= 1:
                norms[("p1", u - 1)]()
        # ---- pass-2 block (uses vas); src-branch projection rides along.
        fill_p2 = {0: [], 1: fosrc[0:3], 2: fosrc[3:CB]}
        for u in range(NH):
            norms[("p2", u)] = attention_pass(
                u, qkst[u], qkt[u], vas, stack_slice(Y01, Y2, u),
                f"p2u{u}", fillers=fill_p2[u], tail=(u == NH - 1))
            if u == 0:
                norms[("p1", 2)]()
            else:
                norms[("p2", u - 1)]()
        norms[("p2", 2)]()
        oproj(Y01, Y2, pj1, pj2, oy_d, "oy", copy_eng="both")

    nc.compile()
    return nc


def _get_program(NT: int) -> "bacc.Bacc":
    if NT not in _PROG_CACHE:
        _PROG_CACHE[NT] = _build_program(NT)
    return _PROG_CACHE[NT]


def make_in_maps(x, src, qk_w, qk_src_w, v_w, v_src_w, proj_w, proj_src_w):
    """Host-side sharding: per-core input dicts (pure data marshalling)."""
    bf = ml_dtypes.bfloat16

    def prep(a):
        return np.ascontiguousarray(a).astype(bf)

    in_maps = []
    for c in range(N_CORES):
        b = c // 4
        heads = [3 * (c % 4) + j for j in range(NH)]
        qk_rows = np.concatenate([qk_w[D2 * h:D2 * (h + 1), :] for h in heads])
        qks_rows = np.concatenate(
            [qk_src_w[D2 * h:D2 * (h + 1), :] for h in heads])
        v_rows = np.concatenate([v_w[DH * h:DH * (h + 1), :] for h in heads])
        vs_rows = np.concatenate(
            [v_src_w[DH * h:DH * (h + 1), :] for h in heads])
        pj_cols = np.concatenate(
            [proj_w[:, DH * h:DH * (h + 1)] for h in heads], axis=1)
        pjs_cols = np.concatenate(
            [proj_src_w[:, DH * h:DH * (h + 1)] for h in heads], axis=1)
        in_maps.append({
            "ident": np.eye(P).astype(ml_dtypes.bfloat16),
            "xT": prep(x[b].T),
            "srcT": prep(src[b].T),
            "qk_wT": prep(qk_rows.T * SCALE),
            "qks_wT": prep(qks_rows.T),
            "v_wT": prep(v_rows.T),
            "vs_wT": prep(vs_rows.T),
            "projT": prep(pj_cols.T),
            "projsT": prep(pjs_cols.T),
        })
    return in_maps


LAST_RESULTS = None  # BassKernelResults of the most recent kernel() call
_HOOK_DONE = False


def _install_ntff_hook():
    """The agent image's antenv lacks axon_hooks; inject a stub module and
    register the ctypes NTFF profile hook so trace=True yields exec times."""
    global _HOOK_DONE
    if _HOOK_DONE:
        return
    try:
        import types
        import antenv  # noqa: F401
        if "antenv.axon_hooks" not in sys.modules:
            mod = types.ModuleType("antenv.axon_hooks")
            _hook = [None]
            mod.set_axon_ntff_profile_hook = lambda h: _hook.__setitem__(0, h)
            mod.get_axon_ntff_profile_hook = lambda: _hook[0]
            sys.modules["antenv.axon_hooks"] = mod
        import trn_agent_boot.trn_boot as _tb
        from antenv.axon_hooks import set_axon_ntff_profile_hook
        set_axon_ntff_profile_hook(
            _tb._ntff_profile_via_ctypes("/opt/axon/libaxon_pjrt.so"))
        _HOOK_DONE = True
    except Exception as e:  # profiling is best-effort
        print(f"ntff hook install failed: {e}", file=sys.stderr)


def kernel(x, src, qk_w, qk_src_w, v_w, v_src_w, proj_w, proj_b,
           proj_src_w, proj_src_b):
    global LAST_RESULTS
    x = np.asarray(x, np.float32)
    src = np.asarray(src, np.float32)
    NT = x.shape[1]

    in_maps = make_in_maps(
        x, src,
        np.asarray(qk_w, np.float32), np.asarray(qk_src_w, np.float32),
        np.asarray(v_w, np.float32), np.asarray(v_src_w, np.float32),
        np.asarray(proj_w, np.float32), np.asarray(proj_src_w, np.float32),
    )

    nc = _get_program(NT)
    trace = bool(int(os.environ.get("BCA_TRACE", "0")))
    if trace:
        _install_ntff_hook()
    res = bass_utils.run_bass_kernel_spmd(
        nc, in_maps, core_ids=list(range(N_CORES)), trace=trace,
    )
    LAST_RESULTS = res

    # host gather: sum partial projections over the 4 cores of each batch,
    # transpose back, add biases, concat branches.
    oy = np.zeros((B, NT, C), np.float32)
    oys = np.zeros((B, NT, C), np.float32)
    for c in range(N_CORES):
        b = c // 4
        oy[b] += np.asarray(res.results[c]["out_y"], np.float32).T
        oys[b] += np.asarray(res.results[c]["out_ys"], np.float32).T
    oy += np.asarray(proj_b, np.float32)
    oys += np.asarray(proj_src_b, np.float32)
    return np.concatenate([oy, oys], axis=-1).astype(np.float32)

